# revision 1
# baseline (speedup 1.0000x reference)
"""Deformable Conv2d on 8 Trainium2 NeuronCores — v2 (row-pair gather).

Sharding: core k -> (batch b = k//2, image row-half yh = k%2).
Each core handles 2048 output pixels (32 rows x 64 cols), all 9 taps,
full C=256 / F=256.

v3: xrp build via HWDGE+ACT (off the SWDGE gather queue).
v2 change vs baseline: the bf16 gather source is a row-pair image
xrp[(y,x)] = concat(x[y,x,:], x[y+1,x,:]) (4 MB DRAM scratch). One
2KB gather descriptor per (pixel, tap) then covers all FOUR bilinear
corners ((y0,x0),(y0,x0+1),(y1,x0),(y1,x0+1)), halving the SWDGE
descriptor count (18432 vs 36864 per core), the gather call count
(18 vs 36), and the index-preparation machinery (one idx plane, not
two).

Per-core device pipeline (bf16 compute, f32 psum accumulation):
  1. build xrp bf16 DRAM scratch from x[b] via 3 casting DMAs.
  2. coords/weights/idx from offsets on DVE (int-convert floor with
     round-up correction; tap grid matches the reference's meshgrid
     quirk). idx = y0*64 + x0 (single plane).
  3. gpsimd.dma_gather per (jh, tap): 1024 descriptors fetch 4C rows
     -> g[128 px, 8 j, 4C].
  4. bilinear blend as 1 activation (ACT) + 3 scalar_tensor_tensor
     (DVE) ops with per-partition weights -> deform[px, c] bf16.
  5. PE transpose deform tiles -> deformT[c, px] (stage-2 lhsT).
  6. 18 accumulating matmuls per 128-pixel tile: out_psum[px, f] +=
     deformT[c,px].T @ W[n][c,f]; copy psum -> f32 out, DMA store.
Bias is added on host during unshard (zeros in this problem).
"""

import numpy as np

B, IH, IW, C = 4, 64, 64, 256
KH, KW, F = 3, 3, 256
N = KH * KW
HALF = IH // 2           # 32 rows per core
PX = HALF * IW           # 2048 pixels per core
NJ = PX // 128           # 16 column-tiles of 128 pixels
NCORES = 8

_cache = {}


def _host_consts(yh):
    # base grid planes [128, N, NJ]: col (n, j), partition p, pixel = j*128+p
    # Tap grid offsets reproduce the reference's meshgrid-stack-reshape quirk.
    flat = np.array([0, 0, 0, 1, 1, 1, 2, 2, 2, 0, 1, 2, 0, 1, 2, 0, 1, 2])
    DY = flat[0::2]
    DX = flat[1::2]
    p = np.arange(128)
    j = np.arange(NJ)
    px = j[None, :] * 128 + p[:, None]          # [128, NJ] local pixel id
    Y = yh * HALF + px // IW                    # global row
    X = px % IW
    baseY = (Y[:, None, :] - 1 + DY[None, :, None]).astype(np.float32)
    baseX = (X[:, None, :] - 1 + DX[None, :, None]).astype(np.float32)
    return baseY.reshape(128, N * NJ), baseX.reshape(128, N * NJ)


def _build_bass(reps=1):
    """Build the per-core kernel. reps>1 unrolls the whole per-call pipeline
    (input loads, coords, xrp build, gather, blend, matmul, store) that many
    times inside one NEFF — used by the timing harness to measure the
    steady-state per-execution time without per-dispatch overhead. Constants
    (W, identity, base grid) load once, as in a serving scenario."""
    import os
    ABL = os.environ.get("BASS_ABLATE", "")
    NQ = int(os.environ.get("BASS_V2_NQ", "1"))        # SWDGE queues
    GCALL = int(os.environ.get("BASS_V2_GCALL", "1024"))  # idxs per gather
    SCR = int(os.environ.get("BASS_V2_SCRATCH", "32768"))
    SP = bool(int(os.environ.get("BASS_V2_SP", "0")))
    CPE = os.environ.get("BASS_V2_COPY", "act")  # deformT copy engine
    import concourse.bass as bass
    import concourse.mybir as mybir
    import concourse.tile as tile
    from concourse import bacc

    from concourse import library_config

    dt = mybir.dt
    Alu = mybir.AluOpType
    # dma_gather descriptors live in the SWDGE ring (size//16 entries carved
    # out of SBUF): 32KB -> 2048 entries; gathers are 1024-index calls so
    # two stay in flight.
    nc = bacc.Bacc(None, target_bir_lowering=False,
                   dynamic_dma_scratch_size=SCR,
                   num_swdge_queues=NQ)

    xin = nc.dram_tensor("x", [IH * IW, C], dt.float32, kind="ExternalInput")
    offs_in = nc.dram_tensor("offs", [PX, 2 * N], dt.float32, kind="ExternalInput")
    w_in = nc.dram_tensor("w", [N, C, F], dt.float32, kind="ExternalInput")
    baseY_in = nc.dram_tensor("baseY", [128, N * NJ], dt.float32, kind="ExternalInput")
    baseX_in = nc.dram_tensor("baseX", [128, N * NJ], dt.float32, kind="ExternalInput")
    ident_in = nc.dram_tensor("ident", [128, 128], dt.bfloat16, kind="ExternalInput")
    out_t = nc.dram_tensor("out", [PX, F], dt.float32, kind="ExternalOutput")

    NPLANE = N * NJ  # 144
    NW2 = N * 128    # wrapped idx plane: 1152 cols

    with tile.TileContext(nc) as tc:
        with tc.tile_pool(name="dram", bufs=2, space="DRAM") as dpool:
            with tc.tile_pool(name="main", bufs=1) as pool:
                nc.gpsimd.load_library(library_config.attnmlp)
                # Warm the Q7 library IRAM (~6us load on first custom inst)
                # during the x-cast window: a minimal gather, result unused.
                warm_idx = pool.tile([128, 8], dt.int16)
                warm_out = pool.tile([128, 1, 64], dt.float32)
                nc.vector.memset(warm_idx[:], 0)
                nc.gpsimd.dma_gather(
                    out_ap=warm_out[:],
                    in_ap=bass.AP(xin, 0, [[64, 128], [1, 64]]),
                    idxs_ap=warm_idx[:],
                    num_idxs=128,
                    num_idxs_reg=128,
                    elem_size=64,
                    elem_step=64,
                )
                # ---- constants / weights ----
                wb = pool.tile([128, N, 2, F], dt.bfloat16)     # Wb[c%128, n, ch, f]
                nc.gpsimd.dma_start(
                    wb[:],
                    bass.AP(w_in, 0, [[F, 128], [128 * F, 2 * N], [1, F]]),
                )
                ident = pool.tile([128, 128], dt.bfloat16)
                nc.sync.dma_start(ident[:], ident_in[:])
                baseY = pool.tile([128, NPLANE], dt.float32)
                baseX = pool.tile([128, NPLANE], dt.float32)
                nc.scalar.dma_start(baseY[:], baseY_in[:])
                nc.scalar.dma_start(baseX[:], baseX_in[:])
                zrow = pool.tile([1, 2 * C], dt.bfloat16)
                nc.vector.memset(zrow[:], 0.0)

                with (
                    tc.tile_pool(name="rp",
                                 bufs=(2 if GCALL <= 1024 else 1)) as rpool,
                    tc.tile_pool(name="dtp", bufs=1) as dtpool,
                    tc.tile_pool(name="gpool", bufs=3) as gpool,
                    tc.tile_pool(name="dpool2", bufs=2) as dfpool,
                    tc.tile_pool(name="pspool", bufs=6, space="PSUM") as pspool,
                    tc.tile_pool(name="opsum", bufs=2, space="PSUM") as opsum,
                    tc.tile_pool(name="ost", bufs=2) as opool,
                ):
                    for rep in range(reps):
                        _one_call(nc, bass, mybir, dt,
                                  ABL + f";nq={NQ};gcall={GCALL};sp={int(SP)};cpe={CPE}", rep,
                                  dpool, pool, rpool, dtpool, gpool, dfpool,
                                  pspool, opsum, opool,
                                  xin, offs_in, out_t,
                                  wb, ident, baseY, baseX, zrow)
    nc.compile()
    return nc


def _one_call(nc, bass, mybir, dt, ABL, rep,
              dpool, pool, rpool, dtpool, gpool, dfpool, pspool, opsum, opool,
              xin, offs_in, out_t, wb, ident, baseY, baseX, zrow):
    """One full kernel execution: input loads, coords, xrp build, gather,
    blend, transpose, matmul, store."""
    Alu = mybir.AluOpType
    NPLANE = N * NJ
    NW2 = N * 128
    opts = dict(kv.split("=") for kv in ABL.split(";")[1:] if "=" in kv)
    NQ = int(opts.get("nq", "1"))
    GCALL = int(opts.get("gcall", "1024"))
    SP = bool(int(opts.get("sp", "1")))
    CPE = opts.get("cpe", "act")
    ABL = ABL.split(";")[0]

    # row-pair image: entry (y,x) = [x[y,x,:], x[y+1,x,:]], plus one zero
    # entry so the (63,63) descriptor's 2KB read stays in bounds.
    xrp_dram = dpool.tile([IH * IW + 1, 2 * C], dt.bfloat16, tag="xrp")
    idx_dram = dpool.tile([16 * NW2], dt.int16, tag="idxd")

    # offsets: [128, j, 18] (partition = px%128)
    offs = rpool.tile([128, NJ, 2 * N], dt.float32, tag="offs")
    nc.scalar.dma_start(
        offs[:],
        bass.AP(offs_in, 0, [[2 * N, 128], [128 * 2 * N, NJ], [1, 2 * N]]),
    )

    # ---- x -> row-pair bf16 DRAM scratch (HWDGE + ACT cast; keeps the
    # SWDGE queue free for gathers so reps can overlap) ----
    # entry e=(y,x) at (part=e//32, r=e%32); e+64 = (part+2, r), so each
    # r-chunk is self-contained for both halves.
    RC = 4  # rows per chunk
    for rc in range(32 // RC):
        cf = rpool.tile([128, RC, C], dt.float32, tag="cf32", name="cf32")
        nc.sync.dma_start(
            cf[:],
            bass.AP(xin, rc * RC * C, [[32 * C, 128], [C, RC], [1, C]]),
        )
        cb = rpool.tile([128, RC, C], dt.bfloat16, tag="cbf", name="cbf")
        nc.scalar.copy(cb[:], cf[:])
        # half1: xrp[e, 0:C] <- x[e]
        nc.sync.dma_start(
            bass.AP(xrp_dram.tensor,
                    xrp_dram[:].offset + rc * RC * 2 * C,
                    [[32 * 2 * C, 128], [2 * C, RC], [1, C]]),
            cb[:],
        )
        # half2: xrp[e, C:2C] <- x[e+64] for e in 0..4031 (parts 0..125)
        nc.scalar.dma_start(
            bass.AP(xrp_dram.tensor,
                    xrp_dram[:].offset + rc * RC * 2 * C + C,
                    [[32 * 2 * C, 126], [2 * C, RC], [1, C]]),
            bass.AP(cb.tensor, cb[:].offset + 2 * cb[:].ap[0][0],
                    [[cb[:].ap[0][0], 126], [C, RC], [1, C]]),
        )
        # half2 tail: entries (63, x) <- x[63, x] (weight-0 row, finite data
        # matching the clamp; parts 126..127)
        nc.scalar.dma_start(
            bass.AP(xrp_dram.tensor,
                    xrp_dram[:].offset + 4032 * 2 * C + rc * RC * 2 * C + C,
                    [[32 * 2 * C, 2], [2 * C, RC], [1, C]]),
            bass.AP(cb.tensor, cb[:].offset + 126 * cb[:].ap[0][0],
                    [[cb[:].ap[0][0], 2], [C, RC], [1, C]]),
        )
    nc.sync.dma_start(
        bass.AP(xrp_dram.tensor, xrp_dram[:].offset + IH * IW * 2 * C,
                [[2 * C, 1], [1, 2 * C]]),
        zrow[:],
    )

    # ---- coordinates / weights / indices (DVE, f32) ----
    def offview(d):
        # [128, (n, j)] view of offs: element (p, n, j) at offs[p, j, 2n+d]
        return bass.AP(offs.tensor, offs[:].offset + d,
                       [[offs[:].ap[0][0], 128], [2, N], [2 * N, NJ]])

    def rtile(tag, dtype=dt.float32):
        return rpool.tile([128, NPLANE], dtype, tag=tag, name=tag)

    cy = rtile("cy")
    cx = rtile("cx")
    fy = rtile("fy")
    fx = rtile("fx")
    y0 = rtile("y0")
    x0 = rtile("x0")
    uy = rtile("uy")
    vx = rtile("vx")
    w00 = rtile("w00")
    w01 = rtile("w01")
    w10 = rtile("w10")
    w11 = rtile("w11")
    idxc = rtile("idxc", dt.int16)
    idf = rtile("idf")
    itmp = rtile("itmp", dt.int32)
    neg = rtile("neg")

    def floor_into(dst_i, dst_f, src):
        # dst_i = int(src) (trunc or round-nearest, HW-dependent);
        # dst_f = frac; fix up if conversion rounded up.
        nc.vector.tensor_copy(itmp[:], src)
        nc.vector.tensor_copy(dst_i[:], itmp[:])
        nc.vector.tensor_tensor(dst_f[:], src, dst_i[:], Alu.subtract)
        nc.vector.tensor_scalar(neg[:], dst_f[:], 0.0, None, Alu.is_lt)
        nc.vector.tensor_tensor(dst_i[:], dst_i[:], neg[:], Alu.subtract)
        nc.vector.tensor_tensor(dst_f[:], dst_f[:], neg[:], Alu.add)

    nc.vector.tensor_tensor(cy[:], baseY[:], offview(0), Alu.add)
    nc.vector.tensor_scalar(cy[:], cy[:], 0.0, float(IH - 1), Alu.max, Alu.min)
    nc.vector.tensor_tensor(cx[:], baseX[:], offview(1), Alu.add)
    nc.vector.tensor_scalar(cx[:], cx[:], 0.0, float(IW - 1), Alu.max, Alu.min)
    floor_into(y0, fy, cy[:])
    floor_into(x0, fx, cx[:])
    nc.vector.tensor_scalar(uy[:], fy[:], -1.0, 1.0, Alu.mult, Alu.add)
    nc.vector.tensor_scalar(vx[:], fx[:], -1.0, 1.0, Alu.mult, Alu.add)
    nc.vector.tensor_tensor(w00[:], uy[:], vx[:], Alu.mult)
    nc.vector.tensor_tensor(w01[:], uy[:], fx[:], Alu.mult)
    nc.vector.tensor_tensor(w10[:], fy[:], vx[:], Alu.mult)
    nc.vector.tensor_tensor(w11[:], fy[:], fx[:], Alu.mult)
    # idx = y0*64 + x0 (exact in f32), cast to int16.
    # idxc col order: (n, j) -> col = n*16 + j
    nc.vector.scalar_tensor_tensor(idf[:], y0[:], float(IW), x0[:],
                                   Alu.mult, Alu.add)
    nc.vector.tensor_copy(idxc[:], idf[:])

    # ---- idx rearrange to wrapped [16, num/16] layout, replicated ----
    # target idxw[q, n*128 + j*8 + a] = idxc[16a+q, n*16+j]
    # step 1: 8 DMAs (per a) SBUF -> DRAM wrapped layout
    for a in range(8):
        nc.scalar.dma_start(
            bass.AP(idx_dram.tensor, idx_dram[:].offset + a,
                    [[NW2, 16], [128, N], [8, NJ]]),
            bass.AP(idxc.tensor,
                    idxc[:].offset + 16 * a * idxc[:].ap[0][0],
                    [[idxc[:].ap[0][0], 16], [NJ, N], [1, NJ]]),
        )
    # step 2: 8 DMAs (per k) DRAM -> SBUF, replicating to all 128 parts
    idxw = rpool.tile([128, NW2], dt.int16, tag="idxw")
    for k in range(8):
        nc.sync.dma_start(
            bass.AP(idxw.tensor,
                    idxw[:].offset + 16 * k * idxw[:].ap[0][0],
                    [[idxw[:].ap[0][0], 16], [1, NW2]]),
            bass.AP(idx_dram.tensor, idx_dram[:].offset, [[NW2, 16], [1, NW2]]),
        )

    # ---- main per-tap pipeline ----
    deformT = dtpool.tile([128, 2, N, NJ, 128], dt.bfloat16, tag="deformT")
    # 4096 rows declared; row 4095's 4C read overhangs into the zero entry
    # (row 4096), which the tensor allocates.
    xview = bass.AP(xrp_dram.tensor, xrp_dram[:].offset,
                    [[2 * C, IH * IW], [1, 4 * C]])

    NJG = GCALL // 128           # j-tiles per gather call
    NHALF = NJ // NJG            # gather calls per tap
    JH = NJG                     # j-tiles per inner block
    gcnt = [0]
    for jh in range(NHALF):
        for n in range(N):
            g = None
            if "gather" not in ABL:
                g = gpool.tile([128, JH, 4 * C], dt.bfloat16, tag="g")
                base = n * 128 + jh * (GCALL // 16)
                nc.gpsimd.dma_gather(
                    out_ap=g[:],
                    in_ap=xview,
                    idxs_ap=idxw[:, base:base + GCALL // 16],
                    num_idxs=JH * 128,
                    num_idxs_reg=JH * 128,
                    elem_size=4 * C,
                    elem_step=2 * C,
                    queue_num=gcnt[0] % NQ,
                    single_packet=SP,
                )
                gcnt[0] += 1
            JB = min(JH, 8)  # blend/transpose block (dfm tile span)
            for jbase in range(0, JH, JB):
                dfm = dfpool.tile([128, JB, C], dt.bfloat16, tag="dfm")
                for jl2 in ([] if "blend" in ABL else range(JB)):
                    jl = jbase + jl2
                    j = jh * JH + jl
                    col = n * NJ + j
                    dv = dfm[:, jl2, :]
                    # op1 on ACT (activation-copy with per-partition scale);
                    # fused MACs on DVE.
                    nc.scalar.activation(
                        dv, g[:, jl, 0:C],
                        mybir.ActivationFunctionType.Copy,
                        scale=w00[:, col:col + 1])
                    # row-pair entry order: [C:2C] is the y-neighbor (y1,x0),
                    # [2C:3C] is the x-neighbor (y0,x1) from the next entry.
                    nc.vector.scalar_tensor_tensor(
                        dv, g[:, jl, C:2 * C], w10[:, col:col + 1], dv,
                        Alu.mult, Alu.add)
                    nc.vector.scalar_tensor_tensor(
                        dv, g[:, jl, 2 * C:3 * C], w01[:, col:col + 1], dv,
                        Alu.mult, Alu.add)
                    nc.vector.scalar_tensor_tensor(
                        dv, g[:, jl, 3 * C:4 * C], w11[:, col:col + 1], dv,
                        Alu.mult, Alu.add)
                for jl2 in ([] if "tpose" in ABL else range(JB)):
                    jl = jbase + jl2
                    j = jh * JH + jl
                    for ch in range(2):
                        pst = pspool.tile([128, 128], dt.bfloat16, tag="pst")
                        nc.tensor.transpose(
                            pst[:], dfm[:, jl2, ch * 128:(ch + 1) * 128],
                            ident[:])
                        if CPE == "pool" or (CPE == "mix" and ch == 1):
                            nc.gpsimd.tensor_copy(deformT[:, ch, n, j, :],
                                                  pst[:])
                        else:
                            nc.scalar.copy(deformT[:, ch, n, j, :], pst[:])

        # ---- stage 2 for this j-half (overlaps next half) ----
        for j in ([] if "mm" in ABL else range(jh * JH, (jh + 1) * JH)):
            pso = opsum.tile([128, F], dt.float32, tag="pso")
            for n2 in range(N):
                for ch in range(2):
                    nc.tensor.matmul(
                        pso[:],
                        lhsT=deformT[:, ch, n2, j, :],
                        rhs=wb[:, n2, ch, :],
                        start=(n2 == 0 and ch == 0),
                        stop=(n2 == N - 1 and ch == 1),
                    )
            osb = opool.tile([128, F], dt.float32, tag="osb")
            nc.scalar.copy(osb[:], pso[:])
            nc.sync.dma_start(
                bass.AP(out_t, j * 128 * F, [[F, 128], [1, F]]),
                osb[:],
            )


def kernel(**inputs):
    from concourse.bass_utils import run_bass_kernel_spmd

    x = np.asarray(inputs["x"], dtype=np.float32)
    offsets = np.asarray(inputs["offsets"], dtype=np.float32)
    W = np.asarray(inputs["W"], dtype=np.float32)
    b = np.asarray(inputs["b"], dtype=np.float32)

    if "nc" not in _cache:
        _cache["nc"] = _build_bass()
    nc = _cache["nc"]

    import ml_dtypes
    ident = np.eye(128).astype(ml_dtypes.bfloat16)

    in_maps = []
    for k in range(NCORES):
        bb, yh = k // 2, k % 2
        bY, bX = _host_consts(yh)
        in_maps.append({
            "x": np.ascontiguousarray(x[bb].reshape(IH * IW, C)),
            "offs": np.ascontiguousarray(
                offsets[bb, yh * HALF:(yh + 1) * HALF].reshape(PX, 2 * N)),
            "w": np.ascontiguousarray(W),
            "baseY": bY, "baseX": bX, "ident": ident,
        })

    import os
    trace = bool(int(os.environ.get("BASS_DEFORM_TRACE", "0")))
    res = run_bass_kernel_spmd(nc, in_maps, core_ids=list(range(NCORES)),
                               trace=trace)
    _cache["last_result"] = res
    out = np.empty((B, IH, IW, F), dtype=np.float32)
    for k in range(NCORES):
        bb, yh = k // 2, k % 2
        out[bb, yh * HALF:(yh + 1) * HALF] = (
            res.results[k]["out"].reshape(HALF, IW, F))
    out += b  # bias (zeros in this problem; exact elementwise add)
    return out

